# revision 5
# baseline (speedup 1.0000x reference)
"""KMeans soft-assignment layer (vq_codebook) for 8x TRN2 NeuronCores.

softmax(-||x-c||^2 / T) over K=512 centroids, T=0.1.

Math: softmax is invariant to the per-row ||x||^2 term, so
logits = (2*x.c - ||c||^2) / T = x @ (20*c)^T - 10*||c||^2.
The -10*||c||^2 row rides the matmul as two extra contraction rows
(fp16 hi + lo split for accuracy) against ones-rows appended to x^T.

Sharding: data-parallel, batch b -> core b. Each core: 32768 tokens.

Per-core structure (256 token-tiles of 128):
  PE   : fp16 matmul [66,128]^T @ [66,512] -> one PSUM bank [128,512] f32
  DVE  : grouped reduce_max over a 2-bank PSUM pair (negate) -> -m [128,2]
  ACT  : exp(logits - m) per tile, fused row-sum accum -> e fp16, s column
  DVE  : grouped reciprocal over 4 tiles' sums -> r [128,4]
  GPSIMD: e * r -> o fp16 (third elementwise engine; DVE/ACT stay on
          max/exp)
  DMA  : one 2 MB contiguous store per 16 tiles (token order permuted on
         host so each 16-tile group is a contiguous [128, 8192] block)

Emission is software-pipelined: exps trail matmul/max by one 4-tile
sgroup, normalizes trail by two, so ACT/DVE/GPSIMD run concurrently.
"""

import sys

sys.path.insert(0, "/opt/trn_rl_repo")

from contextlib import ExitStack

import numpy as np

import concourse.bacc as bacc
import concourse.bass as bass
import concourse.mybir as mybir
import concourse.tile as tile
from concourse.bass_utils import run_bass_kernel_spmd

N_CORES = 8
B, S, D = 8, 32768, 64
K = 512
TEMP = 0.1
P = 128                      # tokens per tile (partition dim)
N_TILES = S // P             # 256 tiles per core
CD = D + 2                   # contraction: 64 x-dims + 2 ones rows (csq hi/lo)
TPG = 16                     # tiles per output-DMA group
N_GD = N_TILES // TPG        # 16 DMA groups per core
SGT = 4                      # tiles per stats sgroup
N_SG = N_TILES // SGT        # 64 sgroups per core
OUT_ROWS = N_GD * P          # 2048
OUT_COLS = TPG * K           # 8192

F16 = mybir.dt.float16
F32 = mybir.dt.float32

_NC_CACHE = {}


def _build_nc(
    repeats=1,
    bufs_in=3,
    bufs_ps=4,
    bufs_e=10,
    bufs_o=3,
    norm_eng="ggg",          # per-tile rotation: g=gpsimd, d=dve, a=act
    **_ignored,
):
    nc = bacc.Bacc(
        "TRN2", target_bir_lowering=False, debug=False, num_devices=N_CORES
    )
    xt = nc.declare_dram_parameter("xt", [CD, S], F16, isOutput=False)
    rh = nc.declare_dram_parameter("rh", [CD, K], F16, isOutput=False)
    out = nc.declare_dram_parameter(
        "out", [OUT_ROWS, OUT_COLS], F16, isOutput=True
    )

    with tile.TileContext(nc) as tc, ExitStack() as ctx:
        const_pool = ctx.enter_context(tc.tile_pool(name="const", bufs=1))
        in_pool = ctx.enter_context(tc.tile_pool(name="xin", bufs=bufs_in))
        psum_pool = ctx.enter_context(
            tc.tile_pool(name="ps", bufs=bufs_ps, space="PSUM")
        )
        e_pool = ctx.enter_context(tc.tile_pool(name="e", bufs=bufs_e))
        o_pool = ctx.enter_context(tc.tile_pool(name="o", bufs=bufs_o))
        stat_pool = ctx.enter_context(tc.tile_pool(name="stat", bufs=12))

        rhs = const_pool.tile([CD, K], F16)
        nc.sync.dma_start(rhs[:], rh[:])

        for _rep in range(repeats):
            xin = {}     # gd -> input tile [CD, P*TPG]
            otile = {}   # gd -> output tile [P, TPG*K]
            pairs = {}   # g -> (ps0, ps1) psum pair tiles [P, 2*K]
            nmt = {}     # g -> -max [P, SGT]
            st = {}      # g -> sums [P, SGT]
            rt = {}      # g -> 1/sums [P, SGT]
            et = {}      # g -> [4 e tiles]

            def stage_in(gd):
                xin[gd] = in_pool.tile([CD, P * TPG], F16, name="xinb")
                nc.scalar.dma_start(
                    xin[gd][:], xt[:, gd * P * TPG : (gd + 1) * P * TPG]
                )

            def stage_mm_max(g):
                gd = g // (TPG // SGT)
                nm = stat_pool.tile([P, SGT], F32, name="nmb")
                nmt[g] = nm
                ps_list = []
                for pr in range(SGT // 2):
                    ps = psum_pool.tile([P, 2 * K], F32, name="psb")
                    ps_list.append(ps)
                    for h in range(2):
                        j = (g % (TPG // SGT)) * SGT + pr * 2 + h
                        nc.tensor.matmul(
                            ps[:, h * K : (h + 1) * K],
                            xin[gd][:, j * P : (j + 1) * P],
                            rhs[:],
                            start=True, stop=True,
                        )
                    nc.vector.tensor_reduce(
                        nm[:, pr * 2 : pr * 2 + 2],
                        ps[:].rearrange("p (g k) -> p g k", g=2),
                        axis=mybir.AxisListType.X,
                        op=mybir.AluOpType.max,
                        negate=True,
                    )
                pairs[g] = ps_list

            def stage_exp(g):
                s = stat_pool.tile([P, SGT], F32, name="sb")
                st[g] = s
                es = []
                for jj in range(SGT):
                    pr, h = jj // 2, jj % 2
                    e = e_pool.tile([P, K], F16, name="eb")
                    es.append(e)
                    nc.scalar.activation(
                        e[:],
                        pairs[g][pr][:, h * K : (h + 1) * K],
                        mybir.ActivationFunctionType.Exp,
                        bias=nmt[g][:, jj : jj + 1],
                        scale=1.0,
                        accum_out=s[:, jj : jj + 1],
                    )
                et[g] = es
                del pairs[g], nmt[g]

            def stage_recip(g):
                r = stat_pool.tile([P, SGT], F32, name="rb")
                rt[g] = r
                nc.vector.reciprocal(r[:], st[g][:])
                del st[g]

            def stage_norm(g):
                gd = g // (TPG // SGT)
                if gd not in otile:
                    otile[gd] = o_pool.tile([P, TPG * K], F16, name="ob")
                o = otile[gd]
                for jj in range(SGT):
                    j = (g % (TPG // SGT)) * SGT + jj
                    eng = norm_eng[(g * SGT + jj) % len(norm_eng)]
                    dst = o[:, j * K : (j + 1) * K]
                    if eng == "g":
                        nc.gpsimd.tensor_scalar_mul(
                            dst, et[g][jj][:], rt[g][:, jj : jj + 1]
                        )
                    elif eng == "d":
                        nc.vector.tensor_scalar_mul(
                            dst, et[g][jj][:], rt[g][:, jj : jj + 1]
                        )
                    else:
                        nc.scalar.activation(
                            dst, et[g][jj][:],
                            mybir.ActivationFunctionType.Copy,
                            scale=rt[g][:, jj : jj + 1],
                        )
                del et[g], rt[g]
                if g % (TPG // SGT) == TPG // SGT - 1:
                    nc.sync.dma_start(
                        out[gd * P : (gd + 1) * P, :], o[:]
                    )
                    del otile[gd]

            SKEW_IN = 2      # input DMA prefetch lead (in sgroups)
            for t in range(N_SG + 2):
                if t < N_SG:
                    if t == 0:
                        stage_in(0)
                    # prefetch upcoming input chunks SKEW_IN sgroups early
                    gd_pf = (t + SKEW_IN) // (TPG // SGT)
                    if (t + SKEW_IN) % (TPG // SGT) == 0 and 0 < gd_pf < N_GD:
                        stage_in(gd_pf)
                    stage_mm_max(t)
                if 1 <= t and t - 1 < N_SG:
                    stage_exp(t - 1)
                if 2 <= t and t - 2 < N_SG:
                    stage_recip(t - 2)
                    stage_norm(t - 2)

    nc.compile()
    return nc


def _prep_inputs(x, centroids):
    x = np.asarray(x, dtype=np.float32)
    centroids = np.asarray(centroids, dtype=np.float32)
    csq = np.sum(centroids.astype(np.float64) ** 2, axis=1)
    hi = (-csq / TEMP).astype(np.float16)
    lo = (-csq / TEMP - hi.astype(np.float64)).astype(np.float16)
    rh = np.empty((CD, K), np.float16)
    rh[0:D] = ((2.0 / TEMP) * centroids.T).astype(np.float16)
    rh[D] = hi
    rh[D + 1] = lo

    in_maps = []
    for b in range(N_CORES):
        xt = np.empty((CD, S), np.float16)
        # permute tokens so each 16-tile DMA group's output is contiguous:
        # tile j of group gd holds tokens gd*2048 + p*16 + j (p = partition)
        xb = x[b].T.astype(np.float16)             # (D, S)
        xb = xb.reshape(D, N_GD, P, TPG).transpose(0, 1, 3, 2)
        xt[0:D] = xb.reshape(D, S)
        xt[D] = 1.0
        xt[D + 1] = 1.0
        in_maps.append({"xt": np.ascontiguousarray(xt), "rh": rh})
    return in_maps


def kernel(x, centroids):
    x = np.asarray(x)
    centroids = np.asarray(centroids)
    in_maps = _prep_inputs(x, centroids)

    if "nc" not in _NC_CACHE:
        _NC_CACHE["nc"] = _build_nc(1)
    nc = _NC_CACHE["nc"]

    res = run_bass_kernel_spmd(nc, in_maps, list(range(N_CORES))).results
    out = np.stack(
        [res[b]["out"].reshape(S, K) for b in range(N_CORES)], axis=0
    )
    return out.reshape(B, S, K).astype(np.float32)


if __name__ == "__main__":
    xs = np.random.randn(B, S, D).astype(np.float32)
    cs = np.random.randn(K, D).astype(np.float32)
    o = kernel(xs, cs)
    print(o.shape, o.dtype, o[0, 0, :4])


# revision 6
# speedup vs baseline: 16.8010x; 16.8010x over previous
"""KMeans soft-assignment layer (vq_codebook) for 8x TRN2 NeuronCores.

softmax(-||x-c||^2 / T) over K=512 centroids, T=0.1.

Math: softmax is invariant to the per-row ||x||^2 term, so
logits = (2*x.c - ||c||^2) / T = x @ (20*c)^T - 10*||c||^2.
The -10*||c||^2 row rides the matmul as two extra contraction rows
(fp16 hi + lo split for accuracy) against ones-rows appended to x^T.

Sharding: data-parallel, batch b -> core b. Each core: 32768 tokens.

Per-core structure (256 token-tiles of 128):
  PE   : fp16 matmul [66,128]^T @ [66,512] -> one PSUM bank [128,512] f32
  DVE  : grouped reduce_max over a 2-bank PSUM pair (negate) -> -m [128,2]
  ACT  : exp(logits - m) per tile, fused row-sum accum -> e fp16, s column
  DVE  : grouped reciprocal over 4 tiles' sums -> r [128,4]
  GPSIMD: e * r -> o fp16 (third elementwise engine; DVE/ACT stay on
          max/exp)
  DMA  : one 2 MB contiguous store per 16 tiles (token order permuted on
         host so each 16-tile group is a contiguous [128, 8192] block)

Emission is software-pipelined: exps trail matmul/max by one 4-tile
sgroup, normalizes trail by two, so ACT/DVE/GPSIMD run concurrently.
"""

import sys

sys.path.insert(0, "/opt/trn_rl_repo")

from contextlib import ExitStack

import numpy as np

import concourse.bacc as bacc
import concourse.bass as bass
import concourse.mybir as mybir
import concourse.tile as tile
from concourse.bass_utils import run_bass_kernel_spmd

N_CORES = 8
B, S, D = 8, 32768, 64
K = 512
TEMP = 0.1
P = 128                      # tokens per tile (partition dim)
N_TILES = S // P             # 256 tiles per core
CD = D + 2                   # contraction: 64 x-dims + 2 ones rows (csq hi/lo)
TPG = 16                     # tiles per output-DMA group
N_GD = N_TILES // TPG        # 16 DMA groups per core
SGT = 4                      # tiles per stats sgroup
N_SG = N_TILES // SGT        # 64 sgroups per core
OUT_ROWS = N_GD * P          # 2048
OUT_COLS = TPG * K           # 8192

F16 = mybir.dt.float16
F32 = mybir.dt.float32

_NC_CACHE = {}


def _build_nc(
    repeats=1,
    bufs_in=3,
    bufs_ps=4,
    bufs_e=16,
    bufs_o=3,
    bufs_stat=16,
    norm_eng="d",            # per-tile rotation: g=gpsimd, d=dve, a=act
    fuse_accum=True,         # row sums via ACT accum_out vs DVE grouped sum
    dma_in_sync=True,        # issue input DMAs on sync (SP) vs scalar (ACT)
    **_ignored,
):
    nc = bacc.Bacc(
        "TRN2", target_bir_lowering=False, debug=False, num_devices=N_CORES
    )
    xt = nc.declare_dram_parameter("xt", [CD, S], F16, isOutput=False)
    rh = nc.declare_dram_parameter("rh", [CD, K], F16, isOutput=False)
    out = nc.declare_dram_parameter(
        "out", [OUT_ROWS, OUT_COLS], F16, isOutput=True
    )

    with tile.TileContext(nc) as tc, ExitStack() as ctx:
        const_pool = ctx.enter_context(tc.tile_pool(name="const", bufs=1))
        in_pool = ctx.enter_context(tc.tile_pool(name="xin", bufs=bufs_in))
        psum_pool = ctx.enter_context(
            tc.tile_pool(name="ps", bufs=bufs_ps, space="PSUM")
        )
        e_pool = ctx.enter_context(tc.tile_pool(name="e", bufs=bufs_e))
        o_pool = ctx.enter_context(tc.tile_pool(name="o", bufs=bufs_o))
        stat_pool = ctx.enter_context(tc.tile_pool(name="stat", bufs=bufs_stat))

        rhs = const_pool.tile([CD, K], F16)
        nc.sync.dma_start(rhs[:], rh[:])

        for _rep in range(repeats):
            xin = {}     # gd -> input tile [CD, P*TPG]
            otile = {}   # gd -> output tile [P, TPG*K]
            pairs = {}   # g -> (ps0, ps1) psum pair tiles [P, 2*K]
            nmt = {}     # g -> -max [P, SGT]
            st = {}      # g -> sums [P, SGT]
            rt = {}      # g -> 1/sums [P, SGT]
            et = {}      # g -> [4 e tiles]

            def stage_in(gd):
                xin[gd] = in_pool.tile([CD, P * TPG], F16, name="xinb")
                eng = nc.sync if dma_in_sync else nc.scalar
                eng.dma_start(
                    xin[gd][:], xt[:, gd * P * TPG : (gd + 1) * P * TPG]
                )

            def stage_mm_max(g):
                gd = g // (TPG // SGT)
                nm = stat_pool.tile([P, SGT], F32, name="nmb")
                nmt[g] = nm
                ps_list = []
                for pr in range(SGT // 2):
                    ps = psum_pool.tile([P, 2 * K], F32, name="psb")
                    ps_list.append(ps)
                    for h in range(2):
                        j = (g % (TPG // SGT)) * SGT + pr * 2 + h
                        nc.tensor.matmul(
                            ps[:, h * K : (h + 1) * K],
                            xin[gd][:, j * P : (j + 1) * P],
                            rhs[:],
                            start=True, stop=True,
                        )
                    nc.vector.tensor_reduce(
                        nm[:, pr * 2 : pr * 2 + 2],
                        ps[:].rearrange("p (g k) -> p g k", g=2),
                        axis=mybir.AxisListType.X,
                        op=mybir.AluOpType.max,
                        negate=True,
                    )
                pairs[g] = ps_list

            def stage_exp(g):
                s = stat_pool.tile([P, SGT], F32, name="sb")
                st[g] = s
                if fuse_accum:
                    es = []
                    for jj in range(SGT):
                        pr, h = jj // 2, jj % 2
                        e = e_pool.tile([P, K], F16, name="eb")
                        es.append(e)
                        nc.scalar.activation(
                            e[:],
                            pairs[g][pr][:, h * K : (h + 1) * K],
                            mybir.ActivationFunctionType.Exp,
                            bias=nmt[g][:, jj : jj + 1],
                            scale=1.0,
                            accum_out=s[:, jj : jj + 1],
                        )
                    et[g] = es
                else:
                    eg = e_pool.tile([P, SGT * K], F16, name="eb")
                    for jj in range(SGT):
                        pr, h = jj // 2, jj % 2
                        nc.scalar.activation(
                            eg[:, jj * K : (jj + 1) * K],
                            pairs[g][pr][:, h * K : (h + 1) * K],
                            mybir.ActivationFunctionType.Exp,
                            bias=nmt[g][:, jj : jj + 1],
                            scale=1.0,
                        )
                    nc.vector.tensor_reduce(
                        s[:],
                        eg[:].rearrange("p (g k) -> p g k", g=SGT),
                        axis=mybir.AxisListType.X,
                        op=mybir.AluOpType.add,
                    )
                    et[g] = eg
                del pairs[g], nmt[g]

            def stage_recip(g):
                r = stat_pool.tile([P, SGT], F32, name="rb")
                rt[g] = r
                nc.vector.reciprocal(r[:], st[g][:])
                del st[g]

            def stage_norm(g):
                gd = g // (TPG // SGT)
                if gd not in otile:
                    otile[gd] = o_pool.tile([P, TPG * K], F16, name="ob")
                o = otile[gd]
                for jj in range(SGT):
                    j = (g % (TPG // SGT)) * SGT + jj
                    eng = norm_eng[(g * SGT + jj) % len(norm_eng)]
                    dst = o[:, j * K : (j + 1) * K]
                    esrc = (et[g][jj][:] if fuse_accum
                            else et[g][:, jj * K : (jj + 1) * K])
                    if eng == "g":
                        nc.gpsimd.tensor_scalar_mul(
                            dst, esrc, rt[g][:, jj : jj + 1]
                        )
                    elif eng == "d":
                        nc.vector.tensor_scalar_mul(
                            dst, esrc, rt[g][:, jj : jj + 1]
                        )
                    else:
                        nc.scalar.activation(
                            dst, esrc,
                            mybir.ActivationFunctionType.Copy,
                            scale=rt[g][:, jj : jj + 1],
                        )
                del et[g], rt[g]
                if g % (TPG // SGT) == TPG // SGT - 1:
                    nc.sync.dma_start(
                        out[gd * P : (gd + 1) * P, :], o[:]
                    )
                    del otile[gd]

            SKEW_IN = 2      # input DMA prefetch lead (in sgroups)
            for t in range(N_SG + 2):
                if t < N_SG:
                    if t == 0:
                        stage_in(0)
                    # prefetch upcoming input chunks SKEW_IN sgroups early
                    gd_pf = (t + SKEW_IN) // (TPG // SGT)
                    if (t + SKEW_IN) % (TPG // SGT) == 0 and 0 < gd_pf < N_GD:
                        stage_in(gd_pf)
                    stage_mm_max(t)
                if 1 <= t and t - 1 < N_SG:
                    stage_exp(t - 1)
                if 2 <= t and t - 2 < N_SG:
                    stage_recip(t - 2)
                    stage_norm(t - 2)

    nc.compile()
    return nc


def _prep_inputs(x, centroids):
    x = np.asarray(x, dtype=np.float32)
    centroids = np.asarray(centroids, dtype=np.float32)
    csq = np.sum(centroids.astype(np.float64) ** 2, axis=1)
    hi = (-csq / TEMP).astype(np.float16)
    lo = (-csq / TEMP - hi.astype(np.float64)).astype(np.float16)
    rh = np.empty((CD, K), np.float16)
    rh[0:D] = ((2.0 / TEMP) * centroids.T).astype(np.float16)
    rh[D] = hi
    rh[D + 1] = lo

    in_maps = []
    for b in range(N_CORES):
        xt = np.empty((CD, S), np.float16)
        # permute tokens so each 16-tile DMA group's output is contiguous:
        # tile j of group gd holds tokens gd*2048 + p*16 + j (p = partition)
        xb = x[b].T.astype(np.float16)             # (D, S)
        xb = xb.reshape(D, N_GD, P, TPG).transpose(0, 1, 3, 2)
        xt[0:D] = xb.reshape(D, S)
        xt[D] = 1.0
        xt[D + 1] = 1.0
        in_maps.append({"xt": np.ascontiguousarray(xt), "rh": rh})
    return in_maps


def kernel(x, centroids):
    x = np.asarray(x)
    centroids = np.asarray(centroids)
    in_maps = _prep_inputs(x, centroids)

    if "nc" not in _NC_CACHE:
        _NC_CACHE["nc"] = _build_nc(1)
    nc = _NC_CACHE["nc"]

    res = run_bass_kernel_spmd(nc, in_maps, list(range(N_CORES))).results
    out = np.stack(
        [res[b]["out"].reshape(S, K) for b in range(N_CORES)], axis=0
    )
    return out.reshape(B, S, K).astype(np.float32)


if __name__ == "__main__":
    xs = np.random.randn(B, S, D).astype(np.float32)
    cs = np.random.randn(K, D).astype(np.float32)
    o = kernel(xs, cs)
    print(o.shape, o.dtype, o[0, 0, :4])


# revision 7
# speedup vs baseline: 17.5276x; 1.0432x over previous
"""KMeans soft-assignment layer (vq_codebook) for 8x TRN2 NeuronCores.

softmax(-||x-c||^2 / T) over K=512 centroids, T=0.1.

Math: softmax is invariant to the per-row ||x||^2 term, so
logits = (2*x.c - ||c||^2) / T = x @ (20*c)^T - 10*||c||^2.
The -10*||c||^2 row rides the matmul as two extra contraction rows
(fp16 hi + lo split for accuracy) against ones-rows appended to x^T.

Sharding: data-parallel, batch b -> core b. Each core: 32768 tokens.

Per-core structure (256 token-tiles of 128):
  PE   : fp16 matmul [66,128]^T @ [66,512] -> one PSUM bank [128,512] f32
  DVE  : grouped reduce_max over a 2-bank PSUM pair (negate) -> -m [128,2]
  ACT  : exp(logits - m) per tile, fused row-sum accum -> e fp16, s column
  DVE  : grouped reciprocal over 4 tiles' sums -> r [128,4]
  GPSIMD: e * r -> o fp16 (third elementwise engine; DVE/ACT stay on
          max/exp)
  DMA  : one 2 MB contiguous store per 16 tiles (token order permuted on
         host so each 16-tile group is a contiguous [128, 8192] block)

Emission is software-pipelined: exps trail matmul/max by one 4-tile
sgroup, normalizes trail by two, so ACT/DVE/GPSIMD run concurrently.
"""

import sys

sys.path.insert(0, "/opt/trn_rl_repo")

from contextlib import ExitStack

import numpy as np

import concourse.bacc as bacc
import concourse.bass as bass
import concourse.mybir as mybir
import concourse.tile as tile
from concourse.bass_utils import run_bass_kernel_spmd

N_CORES = 8
B, S, D = 8, 32768, 64
K = 512
TEMP = 0.1
P = 128                      # tokens per tile (partition dim)
N_TILES = S // P             # 256 tiles per core
CD = D + 2                   # contraction: 64 x-dims + 2 ones rows (csq hi/lo)
TPG = 16                     # tiles per output-DMA group
N_GD = N_TILES // TPG        # 16 DMA groups per core
SGT = 4                      # tiles per stats sgroup
N_SG = N_TILES // SGT        # 64 sgroups per core
OUT_ROWS = N_GD * P          # 2048
OUT_COLS = TPG * K           # 8192

F16 = mybir.dt.float16
F32 = mybir.dt.float32

_NC_CACHE = {}


def _build_nc(
    repeats=1,
    bufs_in=3,
    bufs_ps=4,
    bufs_e=16,
    bufs_o=3,
    bufs_stat=16,
    norm_eng="d",            # per-tile rotation: g=gpsimd, d=dve, a=act
    fuse_accum=True,         # row sums via ACT accum_out vs DVE grouped sum
    dma_in_sync=True,        # issue input DMAs on sync (SP) vs scalar (ACT)
    dma_out_alt=False,       # alternate output DMAs across sync/scalar rings
    **_ignored,
):
    nc = bacc.Bacc(
        "TRN2", target_bir_lowering=False, debug=False, num_devices=N_CORES
    )
    xt = nc.declare_dram_parameter("xt", [CD, S], F16, isOutput=False)
    rh = nc.declare_dram_parameter("rh", [CD, K], F16, isOutput=False)
    out = nc.declare_dram_parameter(
        "out", [OUT_ROWS, OUT_COLS], F16, isOutput=True
    )

    with tile.TileContext(nc) as tc, ExitStack() as ctx:
        const_pool = ctx.enter_context(tc.tile_pool(name="const", bufs=1))
        in_pool = ctx.enter_context(tc.tile_pool(name="xin", bufs=bufs_in))
        psum_pool = ctx.enter_context(
            tc.tile_pool(name="ps", bufs=bufs_ps, space="PSUM")
        )
        e_pool = ctx.enter_context(tc.tile_pool(name="e", bufs=bufs_e))
        o_pool = ctx.enter_context(tc.tile_pool(name="o", bufs=bufs_o))
        stat_pool = ctx.enter_context(tc.tile_pool(name="stat", bufs=bufs_stat))

        rhs = const_pool.tile([CD, K], F16)
        nc.sync.dma_start(rhs[:], rh[:])

        for _rep in range(repeats):
            xin = {}     # gd -> input tile [CD, P*TPG]
            otile = {}   # gd -> output tile [P, TPG*K]
            pairs = {}   # g -> (ps0, ps1) psum pair tiles [P, 2*K]
            nmt = {}     # g -> -max [P, SGT]
            st = {}      # g -> sums [P, SGT]
            rt = {}      # g -> 1/sums [P, SGT]
            et = {}      # g -> [4 e tiles]

            def stage_in(gd):
                xin[gd] = in_pool.tile([CD, P * TPG], F16, name="xinb")
                eng = nc.sync if dma_in_sync else nc.scalar
                eng.dma_start(
                    xin[gd][:], xt[:, gd * P * TPG : (gd + 1) * P * TPG]
                )

            def stage_mm_max(g):
                gd = g // (TPG // SGT)
                nm = stat_pool.tile([P, SGT], F32, name="nmb")
                nmt[g] = nm
                ps_list = []
                for pr in range(SGT // 2):
                    ps = psum_pool.tile([P, 2 * K], F32, name="psb")
                    ps_list.append(ps)
                    for h in range(2):
                        j = (g % (TPG // SGT)) * SGT + pr * 2 + h
                        nc.tensor.matmul(
                            ps[:, h * K : (h + 1) * K],
                            xin[gd][:, j * P : (j + 1) * P],
                            rhs[:],
                            start=True, stop=True,
                        )
                    nc.vector.tensor_reduce(
                        nm[:, pr * 2 : pr * 2 + 2],
                        ps[:].rearrange("p (g k) -> p g k", g=2),
                        axis=mybir.AxisListType.X,
                        op=mybir.AluOpType.max,
                        negate=True,
                    )
                pairs[g] = ps_list

            def stage_exp(g):
                s = stat_pool.tile([P, SGT], F32, name="sb")
                st[g] = s
                if fuse_accum:
                    es = []
                    for jj in range(SGT):
                        pr, h = jj // 2, jj % 2
                        e = e_pool.tile([P, K], F16, name="eb")
                        es.append(e)
                        nc.scalar.activation(
                            e[:],
                            pairs[g][pr][:, h * K : (h + 1) * K],
                            mybir.ActivationFunctionType.Exp,
                            bias=nmt[g][:, jj : jj + 1],
                            scale=1.0,
                            accum_out=s[:, jj : jj + 1],
                        )
                    et[g] = es
                else:
                    eg = e_pool.tile([P, SGT * K], F16, name="eb")
                    for jj in range(SGT):
                        pr, h = jj // 2, jj % 2
                        nc.scalar.activation(
                            eg[:, jj * K : (jj + 1) * K],
                            pairs[g][pr][:, h * K : (h + 1) * K],
                            mybir.ActivationFunctionType.Exp,
                            bias=nmt[g][:, jj : jj + 1],
                            scale=1.0,
                        )
                    nc.vector.tensor_reduce(
                        s[:],
                        eg[:].rearrange("p (g k) -> p g k", g=SGT),
                        axis=mybir.AxisListType.X,
                        op=mybir.AluOpType.add,
                    )
                    et[g] = eg
                del pairs[g], nmt[g]

            def stage_recip(g):
                r = stat_pool.tile([P, SGT], F32, name="rb")
                rt[g] = r
                nc.vector.reciprocal(r[:], st[g][:])
                del st[g]

            def stage_norm(g):
                gd = g // (TPG // SGT)
                if gd not in otile:
                    otile[gd] = o_pool.tile([P, TPG * K], F16, name="ob")
                o = otile[gd]
                for jj in range(SGT):
                    j = (g % (TPG // SGT)) * SGT + jj
                    eng = norm_eng[(g * SGT + jj) % len(norm_eng)]
                    dst = o[:, j * K : (j + 1) * K]
                    esrc = (et[g][jj][:] if fuse_accum
                            else et[g][:, jj * K : (jj + 1) * K])
                    if eng == "g":
                        nc.gpsimd.tensor_scalar_mul(
                            dst, esrc, rt[g][:, jj : jj + 1]
                        )
                    elif eng == "d":
                        nc.vector.tensor_scalar_mul(
                            dst, esrc, rt[g][:, jj : jj + 1]
                        )
                    else:
                        nc.scalar.activation(
                            dst, esrc,
                            mybir.ActivationFunctionType.Copy,
                            scale=rt[g][:, jj : jj + 1],
                        )
                del et[g], rt[g]
                if g % (TPG // SGT) == TPG // SGT - 1:
                    deng = (nc.scalar if (dma_out_alt and gd % 2)
                            else nc.sync)
                    deng.dma_start(
                        out[gd * P : (gd + 1) * P, :], o[:]
                    )
                    del otile[gd]

            SKEW_IN = 2      # input DMA prefetch lead (in sgroups)
            for t in range(N_SG + 2):
                if t < N_SG:
                    if t == 0:
                        stage_in(0)
                    # prefetch upcoming input chunks SKEW_IN sgroups early
                    gd_pf = (t + SKEW_IN) // (TPG // SGT)
                    if (t + SKEW_IN) % (TPG // SGT) == 0 and 0 < gd_pf < N_GD:
                        stage_in(gd_pf)
                    stage_mm_max(t)
                if 1 <= t and t - 1 < N_SG:
                    stage_exp(t - 1)
                if 2 <= t and t - 2 < N_SG:
                    stage_recip(t - 2)
                    stage_norm(t - 2)

    nc.compile()
    return nc


def _prep_inputs(x, centroids):
    x = np.asarray(x, dtype=np.float32)
    centroids = np.asarray(centroids, dtype=np.float32)
    csq = np.sum(centroids.astype(np.float64) ** 2, axis=1)
    hi = (-csq / TEMP).astype(np.float16)
    lo = (-csq / TEMP - hi.astype(np.float64)).astype(np.float16)
    rh = np.empty((CD, K), np.float16)
    rh[0:D] = ((2.0 / TEMP) * centroids.T).astype(np.float16)
    rh[D] = hi
    rh[D + 1] = lo

    in_maps = []
    for b in range(N_CORES):
        xt = np.empty((CD, S), np.float16)
        # permute tokens so each 16-tile DMA group's output is contiguous:
        # tile j of group gd holds tokens gd*2048 + p*16 + j (p = partition)
        xb = x[b].T.astype(np.float16)             # (D, S)
        xb = xb.reshape(D, N_GD, P, TPG).transpose(0, 1, 3, 2)
        xt[0:D] = xb.reshape(D, S)
        xt[D] = 1.0
        xt[D + 1] = 1.0
        in_maps.append({"xt": np.ascontiguousarray(xt), "rh": rh})
    return in_maps


def kernel(x, centroids):
    x = np.asarray(x)
    centroids = np.asarray(centroids)
    in_maps = _prep_inputs(x, centroids)

    if "nc" not in _NC_CACHE:
        _NC_CACHE["nc"] = _build_nc(1)
    nc = _NC_CACHE["nc"]

    res = run_bass_kernel_spmd(nc, in_maps, list(range(N_CORES))).results
    out = np.stack(
        [res[b]["out"].reshape(S, K) for b in range(N_CORES)], axis=0
    )
    return out.reshape(B, S, K).astype(np.float32)


if __name__ == "__main__":
    xs = np.random.randn(B, S, D).astype(np.float32)
    cs = np.random.randn(K, D).astype(np.float32)
    o = kernel(xs, cs)
    print(o.shape, o.dtype, o[0, 0, :4])
